# revision 16
# baseline (speedup 1.0000x reference)
"""Trainium2 Bass kernel for nn_CompLayer_50448685859250 (GNN message passing).

Sharding: nodes split across 8 cores; small params replicated; full ent_emb
resident per-core in HBM for source-row gathers (indirect DMA); per-node
score features AllGathered.
"""

from contextlib import ExitStack

import numpy as np

import concourse.bass as bass
import concourse.bacc as bacc_mod
import concourse.mybir as mybir
from concourse.tile import TileContext
from concourse.masks import make_identity
from concourse.bass import IndirectOffsetOnAxis
from concourse import library_config

F32 = mybir.dt.float32
I32 = mybir.dt.int32
U16 = mybir.dt.uint16
I16 = mybir.dt.int16
AF = mybir.ActivationFunctionType
OP = mybir.AluOpType

NEG_SLOPE = 0.2


class Cfg:
    def __init__(self, n_ent=50000, n_rel=500, deg=32, topk=10, d=256, heads=4,
                 cores=8, tile_n=120, gsz=14, debug=False):
        self.debug = debug
        assert n_ent % cores == 0
        self.n_ent, self.n_rel, self.deg, self.topk = n_ent, n_rel, deg, topk
        self.d, self.heads, self.hd = d, heads, d // heads
        self.cores = cores
        self.own = n_ent // cores
        self.tile_n = tile_n
        assert tile_n % 12 == 0
        self.ch_per_tile = tile_n // 12
        self.t = -(-self.own // tile_n)
        self.pad = self.t * tile_n
        self.gsz = gsz
        self.groups = -(-self.t // gsz)
        self.ce = 12 * topk


def _consts(cfg):
    ce, tk = cfg.ce, cfg.topk
    m01 = np.zeros((ce, 48), np.float32)
    s01 = np.zeros((ce, 12), np.float32)
    for r in range(ce):
        ln = r // tk
        m01[r, 4 * ln:4 * ln + 4] = 1.0
        s01[r, ln] = 1.0
    s01t = np.ascontiguousarray(s01.T)
    kp1 = np.tile(np.arange(1, tk + 1, dtype=np.int16), (128, 1))
    iota = np.tile(np.arange(tk, dtype=np.uint16), (128, 1))
    ext = np.zeros((128, cfg.deg * 16), np.float32)
    for p in range(128):
        ext[p, (np.arange(cfg.deg) * 16) + (p % 16)] = 1.0
    return dict(M01=m01, S01=s01, S01T=s01t, KP1=kp1, IOTA=iota, EXT=ext)


def build_nc(cfg: "Cfg", for_hw: bool = False):
    nc = bacc_mod.Bacc() if for_hw else bass.Bass()
    c = cfg
    H, HD, D, R, TK, CE = c.heads, c.hd, c.d, c.n_rel, c.topk, c.ce
    DEG, TN, CHT = c.deg, c.tile_n, c.ch_per_tile

    ent_emb = nc.declare_dram_parameter("ent_emb", [c.n_ent, D], F32, isOutput=False)
    emb_own = nc.declare_dram_parameter("emb_own", [c.pad, D], F32, isOutput=False)
    src_d = nc.declare_dram_parameter("src", [c.pad, DEG], I32, isOutput=False)
    rel_d = nc.declare_dram_parameter("rel", [c.pad, DEG], I32, isOutput=False)
    W_d = nc.declare_dram_parameter("W", [H, D, HD], F32, isOutput=False)
    Wr_d = nc.declare_dram_parameter("W_r", [H, D, HD], F32, isOutput=False)
    a_d = nc.declare_dram_parameter("a", [H, 3 * HD], F32, isOutput=False)
    rel_emb = nc.declare_dram_parameter("rel_emb", [R, D], F32, isOutput=False)
    nw_d = nc.declare_dram_parameter("neigh_w", [D, D], F32, isOutput=False)
    m01_d = nc.declare_dram_parameter("M01", [CE, 48], F32, isOutput=False)
    s01_d = nc.declare_dram_parameter("S01", [CE, 12], F32, isOutput=False)
    s01t_d = nc.declare_dram_parameter("S01T", [12, CE], F32, isOutput=False)
    kp1_d = nc.declare_dram_parameter("KP1", [128, TK], I16, isOutput=False)
    iota_d = nc.declare_dram_parameter("IOTA", [128, TK], U16, isOutput=False)
    ext_d = nc.declare_dram_parameter("EXT", [128, DEG * 16], F32, isOutput=False)
    out_d = nc.declare_dram_parameter("out", [c.own, D], F32, isOutput=True)

    ss_own_d = nc.dram_tensor("ss_own", [c.own, H], F32)
    ss_all_d = nc.dram_tensor("ss_all", [c.n_ent, H], F32, addr_space="Shared")
    sr4_d = nc.dram_tensor("sr4_tab", [R, H], F32)
    sd_own_d = nc.dram_tensor("sd_own", [c.pad, H], F32)
    dbg = {}
    if c.debug:
        dbg["ss0e"] = nc.dram_tensor("dbg_ss0e", [c.pad, DEG], F32)
        dbg["sr0e"] = nc.dram_tensor("dbg_sr0e", [c.pad, DEG], F32)
        dbg["ssel"] = nc.dram_tensor("dbg_ssel", [c.pad, 16], F32)
        dbg["rsel"] = nc.dram_tensor("dbg_rsel", [c.pad, 16], F32)
        dbg["offs"] = nc.dram_tensor("dbg_offs", [CE, c.t * CHT], I32)
        dbg["s4n"] = nc.dram_tensor("dbg_s4n", [CE, c.t * CHT, 48], F32)
        dbg["at"] = nc.dram_tensor("dbg_at", [128, c.t, 2, CHT * 48], F32)
        dbg["nct"] = nc.dram_tensor("dbg_nct", [128, c.t, 2, TN], F32)

    es = ExitStack()
    with TileContext(nc) as tc, es:
        cst = es.enter_context(tc.tile_pool(name="cst", bufs=1))
        sb = es.enter_context(tc.tile_pool(name="sb", bufs=2))
        sb3 = es.enter_context(tc.tile_pool(name="sb3", bufs=3))
        ps1 = es.enter_context(tc.tile_pool(name="ps1", bufs=1, space="PSUM"))
        ps2 = es.enter_context(tc.tile_pool(name="ps2", bufs=3, space="PSUM"))

        if not for_hw:
            nc.gpsimd.load_library(library_config.local_scatter)

        # ---- persistent allocations up front ----
        sr4T = cst.tile([H, R], F32)
        r0row = cst.tile([128, R], F32)
        ones1 = cst.tile([1, 128], F32)
        wswdq = cst.tile([128, 2, 12], F32)
        relT = cst.tile([128, 2, R], F32)

        # ---- constants ----
        ident = cst.tile([128, 128], F32)
        make_identity(nc, ident)
        m01 = cst.tile([CE, 48], F32); nc.sync.dma_start(m01, m01_d[:, :])
        s01 = cst.tile([CE, 12], F32); nc.sync.dma_start(s01, s01_d[:, :])
        s01t = cst.tile([12, CE], F32); nc.sync.dma_start(s01t, s01t_d[:, :])
        kp1 = cst.tile([128, TK], I16); nc.sync.dma_start(kp1, kp1_d[:, :])
        iota = cst.tile([128, TK], U16); nc.sync.dma_start(iota, iota_d[:, :])
        ext = cst.tile([128, DEG * 16], F32); nc.sync.dma_start(ext, ext_d[:, :])
        nw = cst.tile([128, 2, D], F32)
        nc.sync.dma_start(nw, nw_d.ap().rearrange("(j p) x -> p j x", p=128))
        wpr = cst.tile([128, 2, H, HD], F32)
        wrpr = cst.tile([128, 2, H, HD], F32)
        for j in range(2):
            nc.sync.dma_start(
                wpr[:, j, :, :],
                W_d.ap().rearrange("h (j p) e -> j p h e", p=128)[j])
            nc.sync.dma_start(
                wrpr[:, j, :, :],
                Wr_d.ap().rearrange("h (j p) e -> j p h e", p=128)[j])
        a3e = cst.tile([HD, H, 3], F32)
        nc.sync.dma_start(a3e, a_d.ap().rearrange("h (t e) -> e h t", e=HD))

        # ---- weight prep: wswdq [128, 2, 12] (Ws | Wd | Q) ----
        for j in range(2):
            pw = ps1.tile([128, 12], F32, tag="pww")
            for h in range(H):
                for wsrc in range(2):          # 0: W, 1: W_r
                    pt = ps1.tile([HD, 128], F32, tag="pwt")
                    wt = (wpr if wsrc == 0 else wrpr)[:, j, h, :]
                    nc.tensor.transpose(pt, wt, ident)
                    whtj = sb.tile([HD, 128], F32, tag="whtj")
                    nc.vector.tensor_copy(whtj, pt)
                    if wsrc == 0:
                        for part in range(2):  # Ws (a part 0), Wd (a part 1)
                            nc.tensor.matmul(
                                pw[:, part * H + h:part * H + h + 1],
                                whtj, a3e[:, h, part:part + 1],
                                start=True, stop=True)
                    else:                       # Q (a part 2)
                        nc.tensor.matmul(
                            pw[:, 2 * H + h:2 * H + h + 1],
                            whtj, a3e[:, h, 2:3], start=True, stop=True)
            nc.vector.tensor_copy(wswdq[:, j, :], pw)

        # ---- rel tables ----
        nb = -(-R // 128)
        for b in range(nb):
            pb = min(128, R - b * 128)
            remb = sb.tile([128, D], F32, tag="remb")
            nc.sync.dma_start(remb[:pb], rel_emb[b * 128:b * 128 + pb, :])
            for j in range(2):
                pt = ps1.tile([128, 128], F32, tag="pwt")
                nc.tensor.transpose(pt[:, :pb], remb[:pb, j * 128:(j + 1) * 128],
                                    ident[:pb, :pb])
                nc.vector.tensor_copy(relT[:, j, b * 128:b * 128 + pb],
                                      pt[:, :pb])
        # sr4T [4, R] = Q.T @ rel_emb.T
        psr = ps1.tile([H, R], F32, tag="pww")
        for j in range(2):
            nc.tensor.matmul(psr, wswdq[:, j, 2 * H:3 * H], relT[:, j, :],
                             start=(j == 0), stop=(j == 1))
        nc.vector.tensor_copy(sr4T, psr)
        nc.vector.memset(ones1, 1.0)
        pbr = ps1.tile([128, R], F32, tag="pww")
        nc.tensor.matmul(pbr, ones1, sr4T[0:1, :], start=True, stop=True)
        nc.vector.tensor_copy(r0row, pbr)
        # SR4 DRAM table [R, 4]
        for b in range(nb):
            pb = min(128, R - b * 128)
            pt = ps1.tile([128, H], F32, tag="pwt")
            nc.tensor.transpose(pt[:pb, :], sr4T[:, b * 128:b * 128 + pb],
                                ident[:H, :H])
            sr4sb = sb.tile([128, H], F32, tag="sr4sb")
            nc.vector.tensor_copy(sr4sb[:pb], pt[:pb, :])
            nc.sync.dma_start(sr4_d[b * 128:b * 128 + pb, :], sr4sb[:pb])

        # ---- P1: scan own shard ----
        for t in range(c.t):
            embp = sb3.tile([TN, D], F32, tag="embp")
            nc.sync.dma_start(embp, emb_own[t * TN:(t + 1) * TN, :])
            embT = sb.tile([128, 2, TN], F32, tag="embT")
            for j in range(2):
                pt = ps1.tile([128, TN], F32, tag="pwt")
                nc.tensor.transpose(pt, embp[:, j * 128:(j + 1) * 128],
                                    ident[:TN, :TN])
                nc.vector.tensor_copy(embT[:, j, :], pt)
            pssd = ps1.tile([TN, 12], F32, tag="pww")
            for j in range(2):
                nc.tensor.matmul(pssd, embT[:, j, :], wswdq[:, j, :],
                                 start=(j == 0), stop=(j == 1))
            ssd = sb.tile([TN, 12], F32, tag="ssd")
            nc.vector.tensor_copy(ssd, pssd)
            nc.sync.dma_start(sd_own_d[t * TN:(t + 1) * TN, :],
                              ssd[:, H:2 * H])
            nrows = min(TN, c.own - t * TN)
            if nrows > 0:
                nc.sync.dma_start(ss_own_d[t * TN:t * TN + nrows, :],
                                  ssd[:nrows, :H])

        # ---- P2: AllGather ss ----
        if c.cores > 1:
            nc.gpsimd.collective_compute(
                "AllGather", OP.bypass,
                replica_groups=[list(range(c.cores))],
                ins=[ss_own_d.ap().opt()],
                outs=[ss_all_d.ap().opt()],
            )
        else:
            nc.gpsimd.dma_start(out=ss_all_d[:, :], in_=ss_own_d[:, :])

        # ---- main loop over pipeline groups ----
        for g in range(c.groups):
            t0 = g * c.gsz
            t1 = min(c.t, t0 + c.gsz)
            gch = (t1 - t0) * CHT
            srcrelT2 = sb.tile([TK, c.gsz * TN * 2], F32, tag="srcrelT2")

            # P3: scores, topk, compaction
            for t in range(t0, t1):
                srcT = sb3.tile([128, DEG], I32, tag="srcT")
                relT32 = sb3.tile([128, DEG], I32, tag="relT32")
                nc.sync.dma_start(srcT[:TN], src_d[t * TN:(t + 1) * TN, :])
                nc.sync.dma_start(relT32[:TN], rel_d[t * TN:(t + 1) * TN, :])

                ss0e = sb.tile([TN, DEG], F32, tag="ss0e")
                nc.gpsimd.indirect_dma_start(
                    out=ss0e, out_offset=None, in_=ss_all_d.ap(),
                    in_offset=IndirectOffsetOnAxis(ap=srcT[:TN], axis=0))

                relu16 = sb.tile([128, DEG], U16, tag="relu16")
                nc.vector.memset(relu16, 0)
                nc.vector.tensor_copy(relu16[:TN], relT32[:TN])
                sr0w = sb.tile([128, DEG * 16], F32, tag="sr0w")
                nc.gpsimd.indirect_copy(sr0w, r0row, relu16,
                                        i_know_ap_gather_is_preferred=True)
                sr0m = sb.tile([128, DEG * 16], F32, tag="sr0m")
                nc.vector.tensor_tensor(sr0m, sr0w, ext, op=OP.mult)
                sr0e = sb.tile([128, DEG], F32, tag="sr0e")
                nc.vector.tensor_reduce(
                    sr0e.unsqueeze(2),
                    sr0m.rearrange("p (j r) -> p j r", r=16),
                    axis=mybir.AxisListType.X, op=OP.add)

                if c.debug:
                    nc.sync.dma_start(dbg["ss0e"][t * TN:(t + 1) * TN], ss0e)
                    nc.sync.dma_start(dbg["sr0e"][t * TN:(t + 1) * TN],
                                      sr0e[:TN])
                tsc = sb.tile([TN, DEG], F32, tag="tsc")
                nc.vector.tensor_tensor(tsc, ss0e, sr0e[:TN], op=OP.add)

                mx8a = sb.tile([TN, 8], F32, tag="mx8a")
                ix8a = sb.tile([TN, 8], U16, tag="ix8a")
                nc.vector.max_with_indices(mx8a, ix8a, tsc)
                tsc2 = sb.tile([TN, DEG], F32, tag="tsc2")
                nc.vector.match_replace(tsc2, mx8a, tsc, -3.0e38)
                mx8b = sb.tile([TN, 8], F32, tag="mx8b")
                ix8b = sb.tile([TN, 8], U16, tag="ix8b")
                nc.vector.max_with_indices(mx8b, ix8b, tsc2)

                top10 = sb.tile([128, TK], U16, tag="top10")
                nc.vector.tensor_copy(top10, iota)
                nc.vector.tensor_copy(top10[:TN, 0:8], ix8a)
                nc.vector.tensor_copy(top10[:TN, 8:TK], ix8b[:, 0:TK - 8])

                rank = sb.tile([128, DEG], I16, tag="rank")
                nc.gpsimd.local_scatter(rank, kp1, top10.bitcast(I16),
                                        channels=128, num_elems=DEG,
                                        num_idxs=TK)
                rankf = sb.tile([128, DEG], F32, tag="rankf")
                nc.vector.tensor_copy(rankf, rank)
                nc.vector.tensor_scalar(rankf, rankf, -1.0, None, op0=OP.add)
                rankm1 = sb.tile([128, DEG], I16, tag="rankm1")
                nc.vector.tensor_copy(rankm1, rankf)

                srcu = sb.tile([128, DEG], U16, tag="srcu")
                relu = sb.tile([128, DEG], U16, tag="relu")
                nc.vector.memset(srcu, 0)
                nc.vector.memset(relu, 0)
                nc.vector.tensor_copy(srcu[:TN], srcT[:TN])
                nc.vector.tensor_copy(relu[:TN], relT32[:TN])
                ssel = sb.tile([128, 16], U16, tag="ssel")
                rsel = sb.tile([128, 16], U16, tag="rsel")
                nc.gpsimd.local_scatter(ssel, srcu, rankm1, channels=128,
                                        num_elems=16, num_idxs=DEG)
                nc.gpsimd.local_scatter(rsel, relu, rankm1, channels=128,
                                        num_elems=16, num_idxs=DEG)

                sself = sb.tile([TN, TK], F32, tag="sself")
                rself = sb.tile([TN, TK], F32, tag="rself")
                nc.vector.tensor_copy(sself, ssel[:TN, :TK])
                nc.vector.tensor_copy(rself, rsel[:TN, :TK])
                if c.debug:
                    nc.sync.dma_start(
                        dbg["ssel"][t * TN:(t + 1) * TN, :TK], sself)
                    nc.sync.dma_start(
                        dbg["rsel"][t * TN:(t + 1) * TN, :TK], rself)
                pts = ps1.tile([TK, TN], F32, tag="pww")
                ptr = ps1.tile([TK, TN], F32, tag="pwt")
                nc.tensor.transpose(pts, sself, ident[:TN, :TN])
                nc.tensor.transpose(ptr, rself, ident[:TN, :TN])
                off = (t - t0) * TN * 2
                v3 = srcrelT2[:, off:off + 2 * TN].rearrange(
                    "k (n two) -> k n two", two=2)
                nc.vector.tensor_copy(v3[:, :, 0:1],
                                      pts.unsqueeze(2))
                nc.vector.tensor_copy(v3[:, :, 1:2],
                                      ptr.unsqueeze(2))

            # rearrange selected ids to edge-major layout (12 strided DMAs)
            offsE = sb.tile([CE, c.gsz * CHT, 2], F32, tag="offsE")
            srt3 = srcrelT2.rearrange("k (g x) -> k g x", x=24)
            for ln in range(12):
                nc.sync.dma_start(offsE[TK * ln:TK * (ln + 1), :gch, :],
                                  srt3[:, :gch, 2 * ln:2 * ln + 2])
            offS = sb.tile([CE, c.gsz * CHT], I32, tag="offS")
            offR = sb.tile([CE, c.gsz * CHT], I32, tag="offR")
            nc.vector.tensor_copy(offS[:, :gch], offsE[:, :gch, 0])
            nc.vector.tensor_copy(offR[:, :gch], offsE[:, :gch, 1])

            if c.debug:
                nc.sync.dma_start(
                    dbg["offs"][:, t0 * CHT:t0 * CHT + gch], offS[:, :gch])

            # P5: gather + aggregate
            for t in range(t0, t1):
                lt = t - t0
                osl = offS[:, lt * CHT:(lt + 1) * CHT]
                orl = offR[:, lt * CHT:(lt + 1) * CHT]
                ss4e = sb.tile([CE, CHT, H], F32, tag="ss4e")
                nc.gpsimd.indirect_dma_start(
                    out=ss4e, out_offset=None, in_=ss_all_d.ap(),
                    in_offset=IndirectOffsetOnAxis(ap=osl, axis=0))
                sr4e = sb.tile([CE, CHT, H], F32, tag="sr4e")
                nc.gpsimd.indirect_dma_start(
                    out=sr4e, out_offset=None, in_=sr4_d.ap(),
                    in_offset=IndirectOffsetOnAxis(ap=orl, axis=0))
                gt = sb3.tile([CE, CHT, D], F32, tag="gt")
                nc.gpsimd.indirect_dma_start(
                    out=gt, out_offset=None, in_=ent_emb.ap(),
                    in_offset=IndirectOffsetOnAxis(ap=osl, axis=0))

                # sd -> [12, CHT, 4] at partitions 0..11, then edge layout
                sdr = sb.tile([12, CHT, H], F32, tag="sdr")
                nc.sync.dma_start(
                    sdr,
                    sd_own_d[t * TN:(t + 1) * TN, :]
                    .rearrange("(ch l) h -> l ch h", l=12))
                psd = ps1.tile([CE, CHT * H], F32, tag="p5a")
                for ch in range(CHT):
                    nc.tensor.matmul(psd[:, ch * H:(ch + 1) * H], s01t,
                                     sdr[:, ch, :], start=True, stop=True)
                sde = sb.tile([CE, CHT, H], F32, tag="sde")
                nc.vector.tensor_copy(sde, psd.rearrange("e (ch h) -> e ch h",
                                                         h=H))

                s4 = sb.tile([CE, CHT, H], F32, tag="s4")
                nc.vector.tensor_tensor(s4, ss4e, sr4e, op=OP.add)
                nc.vector.tensor_tensor(s4, s4, sde, op=OP.add)
                nc.vector.scalar_tensor_tensor(
                    s4, s4, NEG_SLOPE, s4, op0=OP.mult, op1=OP.max)
                ex4 = sb.tile([CE, CHT, H], F32, tag="ex4")
                nc.scalar.activation(ex4, s4, AF.Exp)

                pr1 = ps1.tile([12, CHT * H], F32, tag="p5a")
                nc.tensor.matmul(pr1, s01,
                                 ex4.rearrange("e ch h -> e (ch h)"),
                                 start=True, stop=True)
                r1 = sb.tile([12, CHT * H], F32, tag="r1")
                nc.vector.reciprocal(r1, pr1)
                pre = ps1.tile([CE, CHT * H], F32, tag="p5a")
                nc.tensor.matmul(pre, s01t, r1, start=True, stop=True)
                rec = sb.tile([CE, CHT, H], F32, tag="rec")
                nc.vector.tensor_copy(rec, pre.rearrange("e (ch h) -> e ch h",
                                                         h=H))

                nrm = sb.tile([CE, CHT, H], F32, tag="nrm")
                nc.vector.tensor_tensor(nrm, ex4, rec, op=OP.mult)
                s4n = sb.tile([CE, CHT, 48], F32, tag="s4n")
                nc.vector.tensor_tensor(
                    s4n.rearrange("e ch (l h) -> e ch l h", h=H),
                    nrm.unsqueeze(2).to_broadcast([CE, CHT, 12, H]),
                    m01.rearrange("e (l h) -> e l h", h=H).unsqueeze(1)
                       .to_broadcast([CE, CHT, 12, H]),
                    op=OP.mult)

                if c.debug:
                    nc.sync.dma_start(
                        dbg["s4n"][:, t * CHT:(t + 1) * CHT, :],
                        s4n)
                gr = sb3.tile([CE, CHT, D], F32, tag="gr")
                nc.gpsimd.indirect_dma_start(
                    out=gr, out_offset=None, in_=rel_emb.ap(),
                    in_offset=IndirectOffsetOnAxis(ap=orl, axis=0))
                at = sb.tile([128, 2, CHT * 48], F32, tag="at")
                atr = sb.tile([128, 2, CHT * 48], F32, tag="atr")
                for j in range(2):
                    pat = ps2.tile([128, CHT * 48], F32, tag="pk")
                    for ch in range(CHT):
                        nc.tensor.matmul(
                            pat[:, ch * 48:(ch + 1) * 48],
                            gt[:, ch, j * 128:(j + 1) * 128],
                            s4n[:, ch, :], start=True, stop=True)
                    if j == 0:
                        nc.vector.tensor_copy(at[:, j, :], pat)
                    else:
                        nc.scalar.copy(at[:, j, :], pat)
                    patr = ps2.tile([128, CHT * 48], F32, tag="pk")
                    for ch in range(CHT):
                        nc.tensor.matmul(
                            patr[:, ch * 48:(ch + 1) * 48],
                            gr[:, ch, j * 128:(j + 1) * 128],
                            s4n[:, ch, :], start=True, stop=True)
                    if j == 0:
                        nc.vector.tensor_copy(atr[:, j, :], patr)
                    else:
                        nc.scalar.copy(atr[:, j, :], patr)

                if c.debug:
                    nc.sync.dma_start(dbg["at"][:, t, :, :], at)
                nct = sb.tile([128, 2, TN], F32, tag="nct")
                for pair in range(2):
                    pnt = ps2.tile([128, TN], F32, tag="pk")
                    for hh in range(2):
                        h = pair * 2 + hh
                        for j in range(2):
                            nc.tensor.matmul(
                                pnt[hh * HD:(hh + 1) * HD, :],
                                wpr[:, j, h, :],
                                at[:, j, :].rearrange("p (n h) -> p n h",
                                                      h=H)[:, :, h],
                                start=(j == 0), stop=False)
                            nc.tensor.matmul(
                                pnt[hh * HD:(hh + 1) * HD, :],
                                wrpr[:, j, h, :],
                                atr[:, j, :].rearrange("p (n h) -> p n h",
                                                       h=H)[:, :, h],
                                start=False, stop=(j == 1))
                    nc.vector.tensor_copy(nct[:, pair, :], pnt)

                if c.debug:
                    nc.sync.dma_start(dbg["nct"][:, t, :, :], nct)
                pout = ps2.tile([TN, D], F32, tag="pk")
                for j in range(2):
                    nc.tensor.matmul(pout, nct[:, j, :], nw[:, j, :],
                                     start=(j == 0), stop=(j == 1))
                outf = sb.tile([TN, D], F32, tag="outf")
                nc.scalar.activation(outf, pout, AF.Tanh)
                nrows = min(TN, c.own - t * TN)
                if nrows > 0:
                    nc.sync.dma_start(out_d[t * TN:t * TN + nrows, :],
                                      outf[:nrows, :])

    return nc


# ------------------------- host wrapper -------------------------

_CACHE = {}


def build_in_maps(cfg, inputs):
    c = cfg
    ent = np.ascontiguousarray(np.asarray(inputs["ent_emb"], np.float32))
    src = np.ascontiguousarray(np.asarray(inputs["src"]).astype(np.int32))
    rel = np.ascontiguousarray(np.asarray(inputs["rel_id"]).astype(np.int32))
    consts = _consts(cfg)
    maps = []
    for core in range(c.cores):
        lo = core * c.own
        hi = lo + c.own

        def pad(x):
            p = np.zeros((c.pad,) + x.shape[1:], x.dtype)
            p[:c.own] = x[lo:hi]
            return p

        maps.append({
            "ent_emb": ent,
            "emb_own": pad(ent),
            "src": pad(src),
            "rel": pad(rel),
            "W": np.ascontiguousarray(np.asarray(inputs["W"], np.float32)),
            "W_r": np.ascontiguousarray(np.asarray(inputs["W_r"], np.float32)),
            "a": np.ascontiguousarray(np.asarray(inputs["a"], np.float32)),
            "rel_emb": np.ascontiguousarray(
                np.asarray(inputs["rel_emb"], np.float32)),
            "neigh_w": np.ascontiguousarray(
                np.asarray(inputs["neigh_w"], np.float32)),
            "M01": consts["M01"], "S01": consts["S01"], "S01T": consts["S01T"],
            "KP1": consts["KP1"], "IOTA": consts["IOTA"], "EXT": consts["EXT"],
        })
    return maps


def kernel(**inputs) -> np.ndarray:
    from concourse.bass_utils import run_bass_kernel_spmd
    cfg = Cfg()
    if "nc" not in _CACHE:
        nc = build_nc(cfg, for_hw=True)
        nc.compile()
        _CACHE["nc"] = nc
    nc = _CACHE["nc"]
    in_maps = build_in_maps(cfg, inputs)
    res = run_bass_kernel_spmd(nc, in_maps, list(range(cfg.cores)))
    outs = [res.results[i]["out"] for i in range(cfg.cores)]
    return np.concatenate(outs, axis=0)
